# revision 45
# baseline (speedup 1.0000x reference)
"""Trainium2 Bass kernel for nn_LocalRegionLevelLoss (8-core data parallel).

loss = sum_{b,p,r} softmax_r(pos@img^T) * relu(margin + max_n(neg@img^T) - pos@img^T)

Strategy:
  - Pure data parallelism: batch dim (128) sharded 16-per-core across 8 cores.
  - Reduced-precision HBM streams (gate is rel_err < 2e-2; this lands at
    6.7e-3): negatives + img as fp8-e4m3, pos as bf16.  Cuts per-core HBM
    traffic f32 45.6MB -> 11.8MB, and fp8 unlocks MatmulPerfMode.DoubleRow
    (256-deep contraction/pass, 2 fp8 rows per PE cell-cycle) so the PE
    keeps up with the DMA stream.
  - Host-side layout prep: contraction dim D on SBUF partitions,
    partition-major contiguous (negT 5KB/partition/batch descriptors); img
    free dim padded 36->48 because the DoubleRow weight AP k-step must be
    16B-aligned.
  - Per batch: 8 DoubleRow neg-sims matmuls into two psum banks (asymmetric
    384/256 split so the critical-path reduce reads less), 8 bf16xfp8
    pos-sims matmuls, strided reduce_max over the 32 negatives, tiny PE
    transpose; softmax/hinge chain with PSUM-direct operands, negated-max
    DVE reduce, exp+den via one ScalarE activation with accum, num/den
    division deferred out of the loop.
  - The softmax-hinge tail of each batch is software-pipelined TWO batches
    back (emit_tail), with sync=False scheduler edges pinning its PE
    transpose between the next-next batch's ns and pos matmuls: every tail
    dependency is then ~2 periods old and the PE never stalls on the DVE
    reduces.  Both the tail emission point and the edges are empirically
    load-bearing with the tile scheduler.
  - DMA stream order is hand-staged: (img01, nt0 halves, pos01, img23)
    first so the cold PE gets dense work immediately, bulk img/pos slotted
    behind nt2 (issued at b==2, before any reader), nt bufs=14 of runway.
  - Each core emits one f32 partial loss; host sums the 8 partials.
"""

import os
import numpy as np

B, P, NN, R, D = 128, 20, 32, 36, 1024
MARGIN = 0.2
M = 8            # cores
BC = B // M      # batches per core
RP = R + P       # 56 rows of img||pos
RPAD = 48        # R padded so DoubleRow AP k-tile step is 16B-aligned
PPAD = 32        # P padded likewise for the pos weights
NROW = P * NN    # 640 negative rows per batch
DC = D // 128    # 8 chunks of the contraction dim
HALF = 384       # asymmetric neg-sims split: the second (critical-path) reduce reads only 256

_compiled = None


def _build_program():
    from contextlib import ExitStack

    import concourse.tile as tile
    from concourse import bacc, mybir
    from concourse.masks import make_identity
    from concourse.tile import add_dep_helper

    f32 = mybir.dt.float32
    bf16 = mybir.dt.bfloat16
    fp8 = mybir.dt.float8e4
    AX = mybir.AxisListType.X
    AF = mybir.ActivationFunctionType
    OP = mybir.AluOpType

    nc = bacc.Bacc("TRN2", target_bir_lowering=False, debug=False, num_devices=M)

    # Memory-bound kernel: the loss tolerance (2e-2) leaves room to ship the
    # 335MB negatives stream and img as fp8-e4m3 and pos as bf16 (measured
    # end-to-end rel err 6.7e-3), cutting HBM traffic 3.9x vs f32.  fp8 also
    # unlocks MatmulPerfMode.DoubleRow (256-deep contraction per pass, 0.5
    # cycles/row), halving PE time so the DMA stream stays the bottleneck.
    negT = nc.dram_tensor("negT", [BC, 128, DC, NROW], fp8, kind="ExternalInput").ap()
    img8T = nc.dram_tensor("img8T", [128, BC, DC, RPAD], fp8, kind="ExternalInput").ap()
    posT = nc.dram_tensor("posT", [128, BC, DC, P], bf16, kind="ExternalInput").ap()
    out = nc.dram_tensor("partial", [1, 1], f32, kind="ExternalOutput").ap()
    DR = mybir.MatmulPerfMode.DoubleRow

    with tile.TileContext(nc) as tc, ExitStack() as ctx:
        singles = ctx.enter_context(tc.tile_pool(name="singles", bufs=1))
        nbuf = ctx.enter_context(tc.tile_pool(name="nbuf", bufs=14))
        small = ctx.enter_context(tc.tile_pool(name="small", bufs=6))
        ps_ns = ctx.enter_context(tc.tile_pool(name="ps_ns", bufs=2, space="PSUM"))
        ps_sm = ctx.enter_context(tc.tile_pool(name="ps_sm", bufs=2, space="PSUM"))
        ps_tr = ctx.enter_context(tc.tile_pool(name="ps_tr", bufs=2, space="PSUM"))

        ident = singles.tile([R, R], f32)
        make_identity(nc, ident)
        ones = singles.tile([P, 1], f32)
        nc.vector.memset(ones, 1.0)
        numacc = singles.tile([P, BC], f32)
        denacc = singles.tile([P, BC], f32)

        # whole-core img (fp8) + pos (bf16) loads.  DMA issue order is paced
        # with the compute: batch 0's nt arrives in c-pair chunks so the
        # first ns matmul starts as early as possible, and img/pos arrive in
        # per-batch-pair pieces interleaved with the nt stream so the stream
        # rate (~1.7us/batch) stays under the compute rate and no single
        # bulk transfer blocks a needed tile.
        img8a = singles.tile([128, BC, DC, RPAD], fp8)
        posa = singles.tile([128, BC, DC, P], bf16)

        def emit_tail(pb, pmx, psims, pE, after_dve=None):
            # softmax-hinge tail for batch pb, emitted two iterations later:
            # every dependency (both reduce_max halves, the sims copy, E) is
            # long complete, so the PE transpose and the DVE/Scalar chain
            # never stall the engines whatever order the scheduler picks.
            tr_ps = ps_tr.tile([P, R], f32, tag="tr")
            tr_inst = nc.tensor.transpose(tr_ps, pmx, ident)
            # hinge pre-relu = (maxneg + margin) - sims; tr_ps is the single
            # PSUM operand
            hr = small.tile([P, R], f32, tag="hr")
            stt_inst = nc.vector.scalar_tensor_tensor(
                out=hr, in0=tr_ps, scalar=MARGIN, in1=psims,
                op0=OP.add, op1=OP.subtract,
            )
            if after_dve is not None:
                # keep the CURRENT batch's reduce_max pair ahead of this
                # (slack-rich) tail op in the DVE stream: otherwise the
                # scheduler's simulated DVE order lags the maxes a full
                # period and that lag is baked into the semaphore ticks the
                # PE transpose waits on.
                add_dep_helper(after_dve.ins, stt_inst.ins, sync=False,
                               reason="maxes before deferred tail on DVE")
            h = small.tile([P, R], f32, tag="h")
            nc.scalar.activation(h, hr, AF.Relu)
            # W = E * h, num col = sum_r W (gpsimd tensor_tensor measured
            # slower here despite being idle -- port sharing with the DVE)
            w = small.tile([P, R], f32, tag="w")
            nc.vector.tensor_mul(w, pE, h)
            nc.vector.reduce_sum(out=numacc[:, pb : pb + 1], in_=w, axis=AX)
            return tr_inst

        # NOTE: warming the PE with throwaway identity transposes during the
        # DMA wait was tried and measured NEUTRAL-to-WORSE: the HAM clock
        # ramp ignores low-activity 36-deep transposes (batch-0 matmuls
        # still ran at cold rates after 5us of them), and the scheduler
        # front-loaded the dummies, delaying batch 0 by ~2.3us.
        pipe = []
        for b in range(BC):
            nt = nbuf.tile([128, DC, NROW], fp8, tag="nt")
            if b == 0:
                # stream order: batch-0 needs (img01, nt0) first, then the
                # whole img so no later batch ever waits on it, then pos in
                # two pieces slotted between nt transfers.
                # head of the stream: only what batches 0/1 need, plus a
                # small img piece covering batches 2-3, so nt1/nt2 arrive
                # back-to-back and the cold PE gets dense work early.  Bulk
                # img/pos ride behind nt2 (issued at b==2, i.e. before any
                # reader, so the dependency tracker still orders them).
                h0 = DC // 2
                nc.sync.dma_start(out=img8a[:, 0:2], in_=img8T[:, 0:2])
                nc.sync.dma_start(out=nt[:, 0:h0], in_=negT[0][:, 0:h0])
                nc.sync.dma_start(out=nt[:, h0:DC], in_=negT[0][:, h0:DC])
                nc.sync.dma_start(out=posa[:, 0:2], in_=posT[:, 0:2])
                nc.sync.dma_start(out=img8a[:, 2:4], in_=img8T[:, 2:4])
            elif b == 2:
                nc.sync.dma_start(out=nt, in_=negT[b])
                nc.sync.dma_start(out=img8a[:, 4:BC], in_=img8T[:, 4:BC])
                nc.sync.dma_start(out=posa[:, 2:9], in_=posT[:, 2:9])
            else:
                nc.sync.dma_start(out=nt, in_=negT[b])
                if b == 4:
                    nc.sync.dma_start(out=posa[:, 9:BC], in_=posT[:, 9:BC])

            # neg sims, [36 x 640] split into two psum banks; DoubleRow
            # contracts two 128-deep k-subtiles per pass (dim1 = k-pair).
            # c-pair outer / bank inner so both banks' matmuls for one chunk
            # share a single weight load and unlock on one nt chunk arrival.
            ns0 = ps_ns.tile([R, HALF], f32, tag="ns0")
            ns1 = ps_ns.tile([R, NROW - HALF], f32, tag="ns1")
            # pos sims in natural [20 x 36] orientation
            sm_ps = ps_sm.tile([P, R], f32, tag="sm")
            for c in range(DC // 2):
                st, sp = (c == 0), (c == DC // 2 - 1)
                nc.tensor.matmul(
                    ns0, img8a[:, b, 2 * c : 2 * c + 2, 0:R],
                    nt[:, 2 * c : 2 * c + 2, 0:HALF],
                    start=st, stop=sp, perf_mode=DR,
                )
            ns_last = None
            for c in range(DC // 2):
                st, sp = (c == 0), (c == DC // 2 - 1)
                ns_last = nc.tensor.matmul(
                    ns1, img8a[:, b, 2 * c : 2 * c + 2, 0:R],
                    nt[:, 2 * c : 2 * c + 2, HALF:NROW],
                    start=st, stop=sp, perf_mode=DR,
                )

            # max over the 32 negatives of each p -> mx [36, 20].  Issued
            # right after the ns matmuls so the DVE starts as soon as the
            # accumulations close.
            mx = small.tile([R, P], f32, tag="mx")
            nc.vector.reduce_max(
                out=mx[:, 0 : 12],
                in_=ns0.rearrange("r (g n) -> r g n", n=NN),
                axis=AX,
            )
            max1_inst = nc.vector.reduce_max(
                out=mx[:, 12 : P],
                in_=ns1.rearrange("r (g n) -> r g n", n=NN),
                axis=AX,
            )

            # deferred tail for batch b-2: its dependencies are two
            # iterations old, so nothing here stalls an engine.  NOTE: the
            # emission position (between the reduce_max pair and the pos
            # matmuls) is empirically load-bearing -- the tile scheduler
            # keys its engine orders off issue order, and both alternatives
            # (before the maxes / after the pos matmuls) measure 5-12us
            # slower end-to-end.
            tr_inst = None
            if len(pipe) >= 2:
                tr_inst = emit_tail(*pipe.pop(0), after_dve=max1_inst)
                add_dep_helper(ns_last.ins, tr_inst.ins, sync=False,
                               reason="pin tr(b-2) after ns(b)")

            # pos sims
            for c in range(DC):
                st, sp = (c == 0), (c == DC - 1)
                pos_inst = nc.tensor.matmul(
                    sm_ps, posa[:, b, c, :], img8a[:, b, c, 0:R], start=st, stop=sp
                )
                if c == 0 and tr_inst is not None:
                    add_dep_helper(tr_inst.ins, pos_inst.ins, sync=False,
                                   reason="pin pos(b) after tr(b-2)")

            # -max_r sims in one DVE op (negated max reduce)
            nsmax = small.tile([P, 1], f32, tag="nsmax")
            nc.vector.tensor_reduce(
                out=nsmax, in_=sm_ps, axis=AX, op=OP.max, negate=True
            )
            # E = exp(s - smax), den col = sum_r E   (one ScalarE op, PSUM in)
            E = small.tile([P, R], f32, tag="E")
            nc.scalar.activation(
                E, sm_ps, AF.Exp, bias=nsmax, scale=1.0,
                accum_out=denacc[:, b : b + 1],
            )
            # sims PSUM->SBUF so the deferred hinge op can use its single
            # PSUM operand slot on tr_ps (frees sm_ps within this iteration,
            # keeping the psum pools at 8 banks).  On the DVE, not ScalarE:
            # the Scalar EXP->ACT_READ chain is serial and would delay the
            # sm bank release (which gates the next pos matmuls) by ~500ns.
            sims = small.tile([P, R], f32, tag="sims")
            nc.vector.tensor_copy(sims, sm_ps)
            pipe.append((b, mx, sims, E))

        for args_ in pipe:
            emit_tail(*args_)

        # loss partial = sum_{p,b} num/den, division deferred out of the loop
        rden = small.tile([P, BC], f32, tag="rden")
        nc.vector.reciprocal(rden, denacc)
        ratio = small.tile([P, BC], f32, tag="ratio")
        nc.vector.tensor_mul(ratio, numacc, rden)
        total = small.tile([P, 1], f32, tag="total")
        nc.vector.reduce_sum(out=total, in_=ratio, axis=AX)
        fs = ps_tr.tile([1, 1], f32, tag="tr")
        nc.tensor.matmul(fs, total, ones, start=True, stop=True)
        res = small.tile([1, 1], f32, tag="res")
        nc.vector.tensor_copy(res, fs)
        nc.sync.dma_start(out=out, in_=res)

    nc.compile()
    return nc


def _maybe_trace_kwargs():
    """Optional NTFF profiling, enabled via BASS_LRL_TRACE=1 (used by test.py)."""
    if os.environ.get("BASS_LRL_TRACE") != "1":
        return {}
    import contextlib
    import ctypes
    import sys
    import types

    try:
        from antenv.axon_hooks import get_axon_ntff_profile_hook  # noqa: F401
    except ImportError:
        so_path = "/opt/axon/libaxon_pjrt.so"
        lib = ctypes.CDLL(so_path)
        lib.axon_start_nrt_profile.argtypes = [
            ctypes.POINTER(ctypes.c_int64),
            ctypes.c_size_t,
        ]
        lib.axon_start_nrt_profile.restype = ctypes.c_int64
        lib.axon_stop_nrt_profile.argtypes = [ctypes.c_char_p]
        lib.axon_stop_nrt_profile.restype = ctypes.c_int64

        @contextlib.contextmanager
        def _hook(output_dir, device_ids):
            import jax

            jax.devices()
            if device_ids:
                ids = (ctypes.c_int64 * len(device_ids))(*device_ids)
                rc = lib.axon_start_nrt_profile(ids, len(device_ids))
            else:
                rc = lib.axon_start_nrt_profile(None, 0)
            if rc != 0:
                raise RuntimeError(f"axon_start_nrt_profile rc={rc}")
            try:
                yield
            finally:
                n = lib.axon_stop_nrt_profile(str(output_dir).encode())
                if n <= 0:
                    print(f"WARNING: ntff capture wrote {n} files")

        mod = types.ModuleType("antenv.axon_hooks")
        mod.get_axon_ntff_profile_hook = lambda: _hook
        mod.set_axon_ntff_profile_hook = lambda h: None
        sys.modules["antenv.axon_hooks"] = mod

    import concourse.bass_utils as bu

    bu.upload_artifacts = lambda tmpdir: "local://" + tmpdir

    tmpdir = os.environ.get("BASS_LRL_TRACE_DIR", "/root/problem/trace_out")
    import shutil

    shutil.rmtree(tmpdir, ignore_errors=True)
    os.makedirs(tmpdir, exist_ok=True)
    kw = {"trace": True, "tmpdir": tmpdir}
    if os.environ.get("BASS_LRL_TRACE_ALL_CORES") == "1":
        kw["trace_cores"] = list(range(M))
    return kw


def _prep_inputs(img_feats, positives, negatives):
    """Build the per-core D-major, partition-major-contiguous input arrays.

    Casts img + negatives to fp8-e4m3 and pos to bf16 (the PE upconverts
    mixed operands; PSUM accumulation stays f32)."""
    import ml_dtypes

    # img8T layout [128, BC, DC, RPAD]: [p, b, c, r] = img[b, r, c*128+p],
    # r zero-padded 36->48 so the DoubleRow weight AP k-step is 16B-aligned
    im8 = img_feats.astype(ml_dtypes.float8_e4m3)
    im8 = im8.transpose(2, 0, 1).reshape(DC, 128, B, R)  # [c, p, b, r]
    im8 = im8.transpose(1, 2, 0, 3)  # [p, b, c, r] (view)
    im8p = np.zeros((128, B, DC, RPAD), dtype=ml_dtypes.float8_e4m3)
    im8p[:, :, :, :R] = im8
    im8 = im8p
    # posT layout [128, BC, DC, P]: [p, b, c, q] = pos[b, q, c*128+p]
    pst = positives.astype(ml_dtypes.bfloat16)
    pst = pst.transpose(2, 0, 1).reshape(DC, 128, B, P)  # [c, p, b, q]
    pst = pst.transpose(1, 2, 0, 3)  # [p, b, c, q] (view)
    # negT layout [B, 128, DC, NROW]: [b, p, c, j] = neg[b, j, c*128+p]
    ngt = negatives.reshape(B, NROW, D).astype(ml_dtypes.float8_e4m3)
    ngt = ngt.transpose(0, 2, 1)  # [B, D, NROW]
    ngt = ngt.reshape(B, DC, 128, NROW).transpose(0, 2, 1, 3)  # [B, p, c, j]

    in_maps = []
    for c in range(M):
        sl = slice(c * BC, (c + 1) * BC)
        in_maps.append(
            {
                "negT": np.ascontiguousarray(ngt[sl]),
                "img8T": np.ascontiguousarray(im8[:, sl]),
                "posT": np.ascontiguousarray(pst[:, sl]),
            }
        )
    return in_maps


def kernel(img_feats, positives, negatives):
    global _compiled
    from concourse.bass_utils import run_bass_kernel_spmd

    img_feats = np.asarray(img_feats, dtype=np.float32)
    positives = np.asarray(positives, dtype=np.float32)
    negatives = np.asarray(negatives, dtype=np.float32)
    assert img_feats.shape == (B, R, D)
    assert positives.shape == (B, P, D)
    assert negatives.shape == (B, P, NN, D)

    in_maps = _prep_inputs(img_feats, positives, negatives)

    if _compiled is None:
        _compiled = _build_program()
    nc = _compiled

    res = run_bass_kernel_spmd(nc, in_maps, list(range(M)), **_maybe_trace_kwargs())
    if res.exec_time_ns is not None:
        kernel.last_exec_time_ns = res.exec_time_ns
    partials = [np.float64(res.results[c]["partial"][0, 0]) for c in range(M)]
    return np.float32(sum(partials))


kernel.last_exec_time_ns = None



# revision 46
# speedup vs baseline: 1.0266x; 1.0266x over previous
"""Trainium2 Bass kernel for nn_LocalRegionLevelLoss (8-core data parallel).

loss = sum_{b,p,r} softmax_r(pos@img^T) * relu(margin + max_n(neg@img^T) - pos@img^T)

Strategy:
  - Pure data parallelism: batch dim (128) sharded 16-per-core across 8 cores.
  - Reduced-precision HBM streams (gate is rel_err < 2e-2; this lands at
    6.7e-3): negatives + img as fp8-e4m3, pos as bf16.  Cuts per-core HBM
    traffic f32 45.6MB -> 11.8MB, and fp8 unlocks MatmulPerfMode.DoubleRow
    (256-deep contraction/pass, 2 fp8 rows per PE cell-cycle) so the PE
    keeps up with the DMA stream.
  - Host-side layout prep: contraction dim D on SBUF partitions,
    partition-major contiguous (negT 5KB/partition/batch descriptors); img
    free dim padded 36->48 because the DoubleRow weight AP k-step must be
    16B-aligned.
  - Per batch: 8 DoubleRow neg-sims matmuls into two psum banks (asymmetric
    384/256 split so the critical-path reduce reads less), 8 bf16xfp8
    pos-sims matmuls, strided reduce_max over the 32 negatives, tiny PE
    transpose; softmax/hinge chain with PSUM-direct operands, negated-max
    DVE reduce, exp+den via one ScalarE activation with accum, num/den
    division deferred out of the loop.
  - The softmax-hinge tail of each batch is software-pipelined TWO batches
    back (emit_tail), with sync=False scheduler edges pinning its PE
    transpose between the next-next batch's ns and pos matmuls: every tail
    dependency is then ~2 periods old and the PE never stalls on the DVE
    reduces.  Both the tail emission point and the edges are empirically
    load-bearing with the tile scheduler.
  - DMA stream order is hand-staged: (img01, nt0 halves, pos01, img23)
    first so the cold PE gets dense work immediately, bulk img/pos slotted
    behind nt2 (issued at b==2, before any reader), nt bufs=14 of runway.
  - Each core emits one f32 partial loss; host sums the 8 partials.
"""

import os
import numpy as np

B, P, NN, R, D = 128, 20, 32, 36, 1024
MARGIN = 0.2
M = 8            # cores
BC = B // M      # batches per core
RP = R + P       # 56 rows of img||pos
RPAD = 48        # R padded so DoubleRow AP k-tile step is 16B-aligned
PPAD = 32        # P padded likewise for the pos weights
NROW = P * NN    # 640 negative rows per batch
DC = D // 128    # 8 chunks of the contraction dim
HALF = 384       # asymmetric neg-sims split: the second (critical-path) reduce reads only 256

_compiled = None


def _build_program():
    from contextlib import ExitStack

    import concourse.tile as tile
    from concourse import bacc, mybir
    from concourse.masks import make_identity
    from concourse.tile import add_dep_helper

    f32 = mybir.dt.float32
    bf16 = mybir.dt.bfloat16
    fp8 = mybir.dt.float8e4
    AX = mybir.AxisListType.X
    AF = mybir.ActivationFunctionType
    OP = mybir.AluOpType

    nc = bacc.Bacc("TRN2", target_bir_lowering=False, debug=False, num_devices=M)

    # Memory-bound kernel: the loss tolerance (2e-2) leaves room to ship the
    # 335MB negatives stream and img as fp8-e4m3 and pos as bf16 (measured
    # end-to-end rel err 6.7e-3), cutting HBM traffic 3.9x vs f32.  fp8 also
    # unlocks MatmulPerfMode.DoubleRow (256-deep contraction per pass, 0.5
    # cycles/row), halving PE time so the DMA stream stays the bottleneck.
    negT = nc.dram_tensor("negT", [BC, 128, DC, NROW], fp8, kind="ExternalInput").ap()
    img8T = nc.dram_tensor("img8T", [128, BC, DC, RPAD], fp8, kind="ExternalInput").ap()
    posT = nc.dram_tensor("posT", [128, BC, DC, P], bf16, kind="ExternalInput").ap()
    out = nc.dram_tensor("partial", [1, 1], f32, kind="ExternalOutput").ap()
    DR = mybir.MatmulPerfMode.DoubleRow

    with tile.TileContext(nc) as tc, ExitStack() as ctx:
        singles = ctx.enter_context(tc.tile_pool(name="singles", bufs=1))
        nbuf = ctx.enter_context(tc.tile_pool(name="nbuf", bufs=14))
        small = ctx.enter_context(tc.tile_pool(name="small", bufs=6))
        ps_ns = ctx.enter_context(tc.tile_pool(name="ps_ns", bufs=2, space="PSUM"))
        ps_sm = ctx.enter_context(tc.tile_pool(name="ps_sm", bufs=2, space="PSUM"))
        ps_tr = ctx.enter_context(tc.tile_pool(name="ps_tr", bufs=2, space="PSUM"))

        ident = singles.tile([R, R], f32)
        make_identity(nc, ident)
        ones = singles.tile([P, 1], f32)
        nc.vector.memset(ones, 1.0)
        numacc = singles.tile([P, BC], f32)
        denacc = singles.tile([P, BC], f32)

        # whole-core img (fp8) + pos (bf16) loads.  DMA issue order is paced
        # with the compute: batch 0's nt arrives in c-pair chunks so the
        # first ns matmul starts as early as possible, and img/pos arrive in
        # per-batch-pair pieces interleaved with the nt stream so the stream
        # rate (~1.7us/batch) stays under the compute rate and no single
        # bulk transfer blocks a needed tile.
        img8a = singles.tile([128, BC, DC, RPAD], fp8)
        posa = singles.tile([128, BC, DC, P], bf16)

        def emit_tail(pb, pmx, psims, pE, after_dve=None):
            # softmax-hinge tail for batch pb, emitted two iterations later:
            # every dependency (both reduce_max halves, the sims copy, E) is
            # long complete, so the PE transpose and the DVE/Scalar chain
            # never stall the engines whatever order the scheduler picks.
            tr_ps = ps_tr.tile([P, R], f32, tag="tr")
            tr_inst = nc.tensor.transpose(tr_ps, pmx, ident)
            # hinge pre-relu = (maxneg + margin) - sims; tr_ps is the single
            # PSUM operand
            hr = small.tile([P, R], f32, tag="hr")
            stt_inst = nc.vector.scalar_tensor_tensor(
                out=hr, in0=tr_ps, scalar=MARGIN, in1=psims,
                op0=OP.add, op1=OP.subtract,
            )
            if after_dve is not None:
                # keep the CURRENT batch's reduce_max pair ahead of this
                # (slack-rich) tail op in the DVE stream: otherwise the
                # scheduler's simulated DVE order lags the maxes a full
                # period and that lag is baked into the semaphore ticks the
                # PE transpose waits on.
                add_dep_helper(after_dve.ins, stt_inst.ins, sync=False,
                               reason="maxes before deferred tail on DVE")
            h = small.tile([P, R], f32, tag="h")
            nc.scalar.activation(h, hr, AF.Relu)
            # W = E * h, num col = sum_r W (gpsimd tensor_tensor measured
            # slower here despite being idle -- port sharing with the DVE)
            w = small.tile([P, R], f32, tag="w")
            nc.vector.tensor_mul(w, pE, h)
            nc.vector.reduce_sum(out=numacc[:, pb : pb + 1], in_=w, axis=AX)
            return tr_inst

        # NOTE: warming the PE with throwaway identity transposes during the
        # DMA wait was tried and measured NEUTRAL-to-WORSE: the HAM clock
        # ramp ignores low-activity 36-deep transposes (batch-0 matmuls
        # still ran at cold rates after 5us of them), and the scheduler
        # front-loaded the dummies, delaying batch 0 by ~2.3us.
        pipe = []
        for b in range(BC):
            nt = nbuf.tile([128, DC, NROW], fp8, tag="nt")
            if b == 0:
                # stream order: batch-0 needs (img01, nt0) first, then the
                # whole img so no later batch ever waits on it, then pos in
                # two pieces slotted between nt transfers.
                # head of the stream: only what batches 0/1 need, plus a
                # small img piece covering batches 2-3, so nt1/nt2 arrive
                # back-to-back and the cold PE gets dense work early.  Bulk
                # img/pos ride behind nt2 (issued at b==2, i.e. before any
                # reader, so the dependency tracker still orders them).
                h0 = DC // 2
                nc.sync.dma_start(out=img8a[:, 0:2], in_=img8T[:, 0:2])
                nc.sync.dma_start(out=nt[:, 0:h0], in_=negT[0][:, 0:h0])
                nc.sync.dma_start(out=nt[:, h0:DC], in_=negT[0][:, h0:DC])
                nc.sync.dma_start(out=posa[:, 0:2], in_=posT[:, 0:2])
                nc.sync.dma_start(out=img8a[:, 2:4], in_=img8T[:, 2:4])
            elif b == 2:
                nc.sync.dma_start(out=nt, in_=negT[b])
                nc.sync.dma_start(out=img8a[:, 4:BC], in_=img8T[:, 4:BC])
                nc.sync.dma_start(out=posa[:, 2:9], in_=posT[:, 2:9])
            else:
                nc.sync.dma_start(out=nt, in_=negT[b])
                if b == 4:
                    nc.sync.dma_start(out=posa[:, 9:BC], in_=posT[:, 9:BC])

            # neg sims, [36 x 640] split into two psum banks; DoubleRow
            # contracts two 128-deep k-subtiles per pass (dim1 = k-pair).
            # c-pair outer / bank inner so both banks' matmuls for one chunk
            # share a single weight load and unlock on one nt chunk arrival.
            ns0 = ps_ns.tile([R, HALF], f32, tag="ns0")
            ns1 = ps_ns.tile([R, NROW - HALF], f32, tag="ns1")
            # pos sims in natural [20 x 36] orientation
            sm_ps = ps_sm.tile([P, R], f32, tag="sm")
            for c in range(DC // 2):
                st, sp = (c == 0), (c == DC // 2 - 1)
                nc.tensor.matmul(
                    ns0, img8a[:, b, 2 * c : 2 * c + 2, 0:R],
                    nt[:, 2 * c : 2 * c + 2, 0:HALF],
                    start=st, stop=sp, perf_mode=DR,
                )
            ns_last = None
            for c in range(DC // 2):
                st, sp = (c == 0), (c == DC // 2 - 1)
                ns_last = nc.tensor.matmul(
                    ns1, img8a[:, b, 2 * c : 2 * c + 2, 0:R],
                    nt[:, 2 * c : 2 * c + 2, HALF:NROW],
                    start=st, stop=sp, perf_mode=DR,
                )

            # max over the 32 negatives of each p -> mx [36, 20].  Issued
            # right after the ns matmuls so the DVE starts as soon as the
            # accumulations close.
            mx = small.tile([R, P], f32, tag="mx")
            nc.vector.reduce_max(
                out=mx[:, 0 : 12],
                in_=ns0.rearrange("r (g n) -> r g n", n=NN),
                axis=AX,
            )
            max1_inst = nc.vector.reduce_max(
                out=mx[:, 12 : P],
                in_=ns1.rearrange("r (g n) -> r g n", n=NN),
                axis=AX,
            )

            # deferred tail for batch b-2: its dependencies are two
            # iterations old, so nothing here stalls an engine.  NOTE: the
            # emission position (between the reduce_max pair and the pos
            # matmuls) is empirically load-bearing -- the tile scheduler
            # keys its engine orders off issue order, and both alternatives
            # (before the maxes / after the pos matmuls) measure 5-12us
            # slower end-to-end.
            tr_inst = None
            if len(pipe) >= 2:
                tr_inst = emit_tail(*pipe.pop(0), after_dve=max1_inst)
                add_dep_helper(ns_last.ins, tr_inst.ins, sync=False,
                               reason="pin tr(b-2) after ns(b)")

            # pos sims
            for c in range(DC):
                st, sp = (c == 0), (c == DC - 1)
                pos_inst = nc.tensor.matmul(
                    sm_ps, posa[:, b, c, :], img8a[:, b, c, 0:R], start=st, stop=sp
                )
                if c == 0 and tr_inst is not None:
                    add_dep_helper(tr_inst.ins, pos_inst.ins, sync=False,
                                   reason="pin pos(b) after tr(b-2)")

            # -max_r sims in one DVE op (negated max reduce)
            nsmax = small.tile([P, 1], f32, tag="nsmax")
            nc.vector.tensor_reduce(
                out=nsmax, in_=sm_ps, axis=AX, op=OP.max, negate=True
            )
            # E = exp(s - smax), den col = sum_r E   (one ScalarE op, PSUM in)
            E = small.tile([P, R], f32, tag="E")
            nc.scalar.activation(
                E, sm_ps, AF.Exp, bias=nsmax, scale=1.0,
                accum_out=denacc[:, b : b + 1],
            )
            # sims PSUM->SBUF so the deferred hinge op can use its single
            # PSUM operand slot on tr_ps (frees sm_ps within this iteration,
            # keeping the psum pools at 8 banks).  ScalarE: moving this to
            # the DVE (to release the sm bank earlier) measured ~1us worse.
            sims = small.tile([P, R], f32, tag="sims")
            nc.scalar.copy(sims, sm_ps)
            pipe.append((b, mx, sims, E))

        for args_ in pipe:
            emit_tail(*args_)

        # loss partial = sum_{p,b} num/den, division deferred out of the loop
        rden = small.tile([P, BC], f32, tag="rden")
        nc.vector.reciprocal(rden, denacc)
        ratio = small.tile([P, BC], f32, tag="ratio")
        nc.vector.tensor_mul(ratio, numacc, rden)
        total = small.tile([P, 1], f32, tag="total")
        nc.vector.reduce_sum(out=total, in_=ratio, axis=AX)
        fs = ps_tr.tile([1, 1], f32, tag="tr")
        nc.tensor.matmul(fs, total, ones, start=True, stop=True)
        res = small.tile([1, 1], f32, tag="res")
        nc.vector.tensor_copy(res, fs)
        nc.sync.dma_start(out=out, in_=res)

    nc.compile()
    return nc


def _maybe_trace_kwargs():
    """Optional NTFF profiling, enabled via BASS_LRL_TRACE=1 (used by test.py)."""
    if os.environ.get("BASS_LRL_TRACE") != "1":
        return {}
    import contextlib
    import ctypes
    import sys
    import types

    try:
        from antenv.axon_hooks import get_axon_ntff_profile_hook  # noqa: F401
    except ImportError:
        so_path = "/opt/axon/libaxon_pjrt.so"
        lib = ctypes.CDLL(so_path)
        lib.axon_start_nrt_profile.argtypes = [
            ctypes.POINTER(ctypes.c_int64),
            ctypes.c_size_t,
        ]
        lib.axon_start_nrt_profile.restype = ctypes.c_int64
        lib.axon_stop_nrt_profile.argtypes = [ctypes.c_char_p]
        lib.axon_stop_nrt_profile.restype = ctypes.c_int64

        @contextlib.contextmanager
        def _hook(output_dir, device_ids):
            import jax

            jax.devices()
            if device_ids:
                ids = (ctypes.c_int64 * len(device_ids))(*device_ids)
                rc = lib.axon_start_nrt_profile(ids, len(device_ids))
            else:
                rc = lib.axon_start_nrt_profile(None, 0)
            if rc != 0:
                raise RuntimeError(f"axon_start_nrt_profile rc={rc}")
            try:
                yield
            finally:
                n = lib.axon_stop_nrt_profile(str(output_dir).encode())
                if n <= 0:
                    print(f"WARNING: ntff capture wrote {n} files")

        mod = types.ModuleType("antenv.axon_hooks")
        mod.get_axon_ntff_profile_hook = lambda: _hook
        mod.set_axon_ntff_profile_hook = lambda h: None
        sys.modules["antenv.axon_hooks"] = mod

    import concourse.bass_utils as bu

    bu.upload_artifacts = lambda tmpdir: "local://" + tmpdir

    tmpdir = os.environ.get("BASS_LRL_TRACE_DIR", "/root/problem/trace_out")
    import shutil

    shutil.rmtree(tmpdir, ignore_errors=True)
    os.makedirs(tmpdir, exist_ok=True)
    kw = {"trace": True, "tmpdir": tmpdir}
    if os.environ.get("BASS_LRL_TRACE_ALL_CORES") == "1":
        kw["trace_cores"] = list(range(M))
    return kw


def _prep_inputs(img_feats, positives, negatives):
    """Build the per-core D-major, partition-major-contiguous input arrays.

    Casts img + negatives to fp8-e4m3 and pos to bf16 (the PE upconverts
    mixed operands; PSUM accumulation stays f32)."""
    import ml_dtypes

    # img8T layout [128, BC, DC, RPAD]: [p, b, c, r] = img[b, r, c*128+p],
    # r zero-padded 36->48 so the DoubleRow weight AP k-step is 16B-aligned
    im8 = img_feats.astype(ml_dtypes.float8_e4m3)
    im8 = im8.transpose(2, 0, 1).reshape(DC, 128, B, R)  # [c, p, b, r]
    im8 = im8.transpose(1, 2, 0, 3)  # [p, b, c, r] (view)
    im8p = np.zeros((128, B, DC, RPAD), dtype=ml_dtypes.float8_e4m3)
    im8p[:, :, :, :R] = im8
    im8 = im8p
    # posT layout [128, BC, DC, P]: [p, b, c, q] = pos[b, q, c*128+p]
    pst = positives.astype(ml_dtypes.bfloat16)
    pst = pst.transpose(2, 0, 1).reshape(DC, 128, B, P)  # [c, p, b, q]
    pst = pst.transpose(1, 2, 0, 3)  # [p, b, c, q] (view)
    # negT layout [B, 128, DC, NROW]: [b, p, c, j] = neg[b, j, c*128+p]
    ngt = negatives.reshape(B, NROW, D).astype(ml_dtypes.float8_e4m3)
    ngt = ngt.transpose(0, 2, 1)  # [B, D, NROW]
    ngt = ngt.reshape(B, DC, 128, NROW).transpose(0, 2, 1, 3)  # [B, p, c, j]

    in_maps = []
    for c in range(M):
        sl = slice(c * BC, (c + 1) * BC)
        in_maps.append(
            {
                "negT": np.ascontiguousarray(ngt[sl]),
                "img8T": np.ascontiguousarray(im8[:, sl]),
                "posT": np.ascontiguousarray(pst[:, sl]),
            }
        )
    return in_maps


def kernel(img_feats, positives, negatives):
    global _compiled
    from concourse.bass_utils import run_bass_kernel_spmd

    img_feats = np.asarray(img_feats, dtype=np.float32)
    positives = np.asarray(positives, dtype=np.float32)
    negatives = np.asarray(negatives, dtype=np.float32)
    assert img_feats.shape == (B, R, D)
    assert positives.shape == (B, P, D)
    assert negatives.shape == (B, P, NN, D)

    in_maps = _prep_inputs(img_feats, positives, negatives)

    if _compiled is None:
        _compiled = _build_program()
    nc = _compiled

    res = run_bass_kernel_spmd(nc, in_maps, list(range(M)), **_maybe_trace_kwargs())
    if res.exec_time_ns is not None:
        kernel.last_exec_time_ns = res.exec_time_ns
    partials = [np.float64(res.results[c]["partial"][0, 0]) for c in range(M)]
    return np.float32(sum(partials))


kernel.last_exec_time_ns = None

